# revision 20
# baseline (speedup 1.0000x reference)
"""CLIF spiking-neuron recurrence kernel for 8 Trainium2 NeuronCores.

Reference semantics (per element, T=64 sequential steps, gamma=0.5):
    u     = 0.5*u + x_t
    spike = (u >= 1.0)
    m     = s_prev * sigmoid(0.5*u) + spike
    s     = sigmoid(m)                       # carried (in-place sigmoid_)
    u     = u - spike*(1.0 + s)
Output: spikes [T, B, D] float32.

Design (vs the 3-group concat baseline at ~162us; this runs ~144us):
- 2 groups of 256 cols; per core [128 x 512] elements. V_g = 2^t*u in
  PSUM (half a bank per group); m in SBUF - no matmul ever touches m, so
  the M2 DVE write never orders against the PE accumulation group (the
  tile tracker is whole-tile for PSUM acc groups; learned the hard way).
- Critical cycle per group: sg-ACT (sigmoid of V) -> Y3 (DVE) -> reset
  matmul -> next sg-ACT. The m/s path (M2 -> s-ACT) runs off-cycle with
  a full step of slack.
- DVE: CLIF_Y3 (y = spike*quad(q), q = s_prev*sg, exact spike via the
  monotone sigmoid LUT compare sg >= c) and CLIF_M2 (m = q + spike).
- Both matmul kinds of step t share ONE stationary W_t = -B0P*2^t*I
  (bf16, exact): the reset (moving y fp16) and the x-add for t+1 (moving
  fp16 hi/lo pair of -2x/B0P, exact to ~2^-21) - weights stay loaded.
- Output is y (fp16, 128KB/step instead of 256KB): host computes
  spike = (y != 0), exact since y = spike*quad and quad >= 1.
"""

import sys
import types

import numpy as np
import ml_dtypes

# If BASS_TRACE is set but the image's antenv lacks axon_hooks,
# run_bass_kernel_spmd would crash importing it; install a null-hook
# module so tracing degrades gracefully instead.
try:
    import antenv.axon_hooks  # noqa: F401
except Exception:
    try:
        import antenv
        _hooks = types.ModuleType("antenv.axon_hooks")
        _hook_cell = [None]
        _hooks.set_axon_ntff_profile_hook = (
            lambda h: _hook_cell.__setitem__(0, h))
        _hooks.get_axon_ntff_profile_hook = lambda: _hook_cell[0]
        sys.modules["antenv.axon_hooks"] = _hooks
        antenv.axon_hooks = _hooks
    except Exception:
        pass

import concourse.bass as bass
import concourse.bacc as bacc
import concourse.mybir as mybir
import concourse.tile as tile
import concourse.dve_ops as dve_ops
from concourse.dve_spec import Spec, Src0, Src1, C0, C1, C2, One, lower, _has_src1
from concourse.dve_uop import DveOpSpec
from concourse.bass_utils import run_bass_kernel_spmd

F32 = mybir.dt.float32
BF16 = mybir.dt.bfloat16
FP16 = mybir.dt.float16
AF = mybir.ActivationFunctionType

T = 64
B = 128
D = 4096
N_CORES = 8
P = 128
NPC = B * D // N_CORES          # 65536 elements per core
FDT = NPC // P                  # 512 free columns per core
GW = 256                        # group width (2 groups)

# Constrained LS fit of f(q) = 1 + sigmoid(1 + q) on q in [0.30, 0.93]
# with constant term forced to B0P (for t>=1 spiking elements always have
# q >= 0.311 since s_prev >= 0.5 and sg >= c; t=0 uses a zero-s path with
# the exact-B0 stationary).
B0P = 1.734375                          # bf16-exact
B1C = 0.18530899
B2C = -0.03826911
YC1 = float(np.float32(B1C / B0P))
YC2 = float(np.float32(B2C / B0P))
B0_EXACT = float(np.float32(1.0 + 1.0 / (1.0 + np.exp(-1.0))))

_NC_CACHE = None
LAST_RESULTS = None


def _register_dve_op(name, spec):
    for op in dve_ops.OPS:
        if op.name == name:
            return op
    shas = {}
    for ver in ("v3", "v4"):
        u = lower(spec, ver=ver)
        shas[ver] = DveOpSpec(name=name, opcode=1, uops=u,
                              rd1_en=_has_src1(spec)).sha(ver)
    op = dve_ops.DveOp(name, spec, subdim=False, uops_sha=shas)
    dve_ops.OPS.append(op)
    dve_ops._SUB_OPCODE_FOR_NAME[name] = (
        dve_ops._CUSTOM_DVE_ROW_BASE + len(dve_ops.OPS) - 1)
    dve_ops.CUSTOM_DVE_SPECS[name] = spec
    return op


# y = (sg >= c) * ((C2*q + C1)*q + 1),  q = s_prev*sg: reset magnitude
# (1+sigmoid(1+q))/B0P as a quadratic of q. in0=s_prev, in1=sg, s0=c,
# s1=YC1, imm2=YC2. (Same op as the proven baseline CLIF_Y3.)
_q = Src0 * Src1
CLIF_Y3 = _register_dve_op("CLIF_Y3_ANT", Spec(
    body=(Src1 >= C0) * ((C2 * _q + C1) * _q + One),
    reference=lambda in0, in1, s0, s1, imm2:
        (in1 >= s0).astype(np.float32)
        * ((imm2 * (in0 * in1) + s1) * (in0 * in1) + 1.0),
))
# m = (s_prev*sg + (sg >= c)) * C1   (C1 = 1; fp32 to SBUF; the s-ACT
# reads it with scale 1 to produce s_t = sigmoid(m))
CLIF_M2 = _register_dve_op("CLIF_M2_ANT", Spec(
    body=(Src0 * Src1 + (Src1 >= C0)) * C1,
    reference=lambda in0, in1, s0, s1, imm2:
        (in0 * in1 + (in1 >= s0).astype(np.float32)) * s1,
))


def _build():
    nc = bacc.Bacc(None, target_bir_lowering=False, debug=False,
                   num_devices=N_CORES)

    # xs[t] = fp16 [hi(512) | lo(512)] pair of the scaled input (see host)
    xs = nc.declare_dram_parameter("xs", [T, P, 2 * FDT], FP16, isOutput=False)
    # wts[:, t, :] = -B0P * 2^t * I  (bf16, exact) for t >= 1
    wts = nc.declare_dram_parameter("wts", [P, T, P], BF16, isOutput=False)
    # w0 = -B0_EXACT * I (fp16)
    w0 = nc.declare_dram_parameter("w0", [P, P], FP16, isOutput=False)
    out = nc.declare_dram_parameter("out", [T, P, FDT], FP16, isOutput=True)

    with tile.TileContext(nc) as tc:
        with (
            tc.tile_pool(name="wpool", bufs=1) as wpool,
            tc.tile_pool(name="cpool", bufs=1) as cpool,
            tc.tile_pool(name="xpool", bufs=6) as xpool,
            tc.tile_pool(name="sgpool", bufs=6) as sgpool,
            tc.tile_pool(name="spool", bufs=4) as spool,
            tc.tile_pool(name="mppool", bufs=4) as mppool,
            tc.tile_pool(name="ypool", bufs=4) as ypool,
            tc.tile_pool(name="vpool", bufs=1, space="PSUM") as vpool,
        ):
            # --- one-time setup -------------------------------------------
            w0t = wpool.tile([P, P], FP16, tag="w0")
            nc.scalar.dma_start(w0t[:], w0[:])
            # Stream the 2MB of (mostly-zero diagonal) weights in 8-step
            # chunks: chunk 0 up front, the rest issued inside the loop a
            # full 8 steps ahead - keeps the prologue off the DMA critical
            # path (this was ~8us of startup stall when loaded up front).
            wtile = wpool.tile([P, T, P], BF16, tag="wts")
            nc.scalar.dma_start(wtile[:, 0:8, :], wts[:, 0:8, :])

            halft = cpool.tile([P, 1], F32, tag="half")
            nc.gpsimd.memset(halft[:], 0.5)
            ct = cpool.tile([P, 1], F32, tag="c")
            # c = sigmoid_LUT(0.5), same LUT as the per-step sigmoids
            nc.scalar.activation(ct[:], halft[:], AF.Sigmoid, bias=0.0, scale=1.0)
            c_ap = ct[:, 0:1]

            zs = cpool.tile([P, FDT], F32, tag="zs")   # zero s_prev for t=0
            nc.gpsimd.memset(zs[:], 0.0)

            # Per-group V state, half a PSUM bank each. m lives in SBUF
            # (mppool) - no matmul ever touches it, so the M2 DVE write
            # never orders against the PE accumulation group.
            V = [vpool.tile([P, GW], F32, name=f"V{g}", tag=f"V{g}")
                 for g in range(2)]

            # PE warm-up: dummy matmuls fill the otherwise-idle prologue
            # window so the HAM clock gate reaches 2.4 GHz before the first
            # real matmul
            junk = vpool.tile([P, 128], F32, tag="junk")
            for _ in range(10):
                nc.tensor.matmul(junk[:], w0t[:], w0t[:], start=True, stop=True,
                                 skip_group_check=True)

            # V_0 = W_0 @ x~_0  (x~_0 = -x_0/B0_EXACT)
            x0 = xpool.tile([P, 2, FDT], FP16, tag="x")
            nc.sync.dma_start(x0[:], xs[0])
            for g in range(2):
                o = g * GW
                nc.tensor.matmul(V[g][:], w0t[:], x0[:, 0, o:o + GW],
                                 start=True, stop=False, skip_group_check=True)
                nc.tensor.matmul(V[g][:], w0t[:], x0[:, 1, o:o + GW],
                                 start=False, stop=False, skip_group_check=True)

            s_prev = zs   # [P, FDT]: s_{t-1} for both groups (zero at t=0)

            # --- the recurrence -------------------------------------------
            for t in range(T):
                sc = float(2.0 ** (-t - 1))
                wsrc = w0t[:] if t == 0 else wtile[:, t, :]
                last = t == T - 1

                # input prefetch for the NEXT step (one wide DMA)
                if t < T - 1:
                    xnext = xpool.tile([P, 2, FDT], FP16, tag="x")
                    nc.sync.dma_start(xnext[:], xs[t + 1])
                # weight chunk for steps [8k, 8k+8), issued 8 steps ahead
                # from the gpsimd queue - the scalar (ACT) queue is the
                # binding engine and must not carry DMA-issue work
                if t % 8 == 0 and t + 8 < T:
                    k = t + 8
                    nc.gpsimd.dma_start(wtile[:, k:k + 8, :], wts[:, k:k + 8, :])

                # sg = sigmoid(2^-(t+1) V) per group (critical-path ACT)
                sgw = []
                for g in range(2):
                    sg_g = sgpool.tile([P, GW], F32, name=f"sg{g}", tag=f"sg{g}")
                    nc.scalar.activation(sg_g[:], V[g][:],
                                         AF.Sigmoid, bias=0.0, scale=sc)
                    sgw.append(sg_g)

                yt = ypool.tile([P, FDT], FP16, tag="y")
                mp = mppool.tile([P, FDT], F32, tag="mp")
                st = spool.tile([P, FDT], F32, tag="st")

                # x-add for t+1 (early: fills PE, keeps the clock ramped).
                # One matmul per group: moving [2, GW] (hi|lo pair), output
                # visits V_g twice via a stride-0 broadcast AP - the PSUM
                # accumulate adds both passes.
                if not last:
                    for g in range(2):
                        o = g * GW
                        vb = V[g][:].unsqueeze(1).broadcast_to([P, 2, GW])
                        nc.tensor.matmul(vb, wsrc, xnext[:, :, o:o + GW],
                                         start=False, stop=False,
                                         skip_group_check=True)

                # DVE order: Y3_g0, M2_g0, Y3_g1, M2_g1 (keeps group 0's
                # whole chain ahead of group 1's)
                for g in range(2):
                    o = g * GW
                    nc.vector._custom_dve(CLIF_Y3, out=yt[:, o:o + GW],
                                          in0=s_prev[:, o:o + GW],
                                          in1=sgw[g][:],
                                          s0=c_ap, s1=YC1, imm2=YC2)
                    if not last:
                        # reset: V_g += W_t @ y_g
                        nc.tensor.matmul(V[g][:], wsrc, yt[:, o:o + GW],
                                         start=False, stop=(t == T - 2),
                                         skip_group_check=True)
                        # m = q + spike (fp32, SBUF)
                        nc.vector._custom_dve(CLIF_M2, out=mp[:, o:o + GW],
                                              in0=s_prev[:, o:o + GW],
                                              in1=sgw[g][:],
                                              s0=c_ap, s1=1.0)
                        # s_t = sigmoid(m) for the next step (off-cycle ACT)
                        nc.scalar.activation(st[:, o:o + GW], mp[:, o:o + GW],
                                             AF.Sigmoid, bias=0.0, scale=1.0)

                # output y_t (fp16); host: spike = (y != 0)
                nc.gpsimd.dma_start(out[t], yt[:])
                if not last:
                    s_prev = st

    nc.compile()
    return nc


def _get_nc():
    global _NC_CACHE
    if _NC_CACHE is None:
        _NC_CACHE = _build()
    return _NC_CACHE


def kernel(x_seq: np.ndarray) -> np.ndarray:
    global LAST_RESULTS
    x = np.ascontiguousarray(x_seq, dtype=np.float32)
    assert x.shape == (T, B, D), x.shape

    # Host prep: x~_t scaled so that the matmul under the stationary W
    # current at issue time yields exactly the 2^t * x_t that V needs
    # (V_t = V_{t-1} + 2^t * x_t):
    #   t=0:  V_0 = W_0 @ x~_0,      W_0 = -B0_EXACT*I -> x~_0 = -x/B0_EXACT
    #   t=1:  issued under W_0                         -> x~_1 = -2x/B0_EXACT
    #   t>=2: issued under W_{t-1} = -B0P*2^{t-1}*I    -> x~_t = -2x/B0P
    xf = x.reshape(T, -1).astype(np.float64)
    scale = np.empty((T, 1), dtype=np.float64)
    scale[0, 0] = -1.0 / B0_EXACT
    scale[1, 0] = -2.0 / B0_EXACT
    scale[2:, 0] = -2.0 / B0P
    xsc = (xf * scale).astype(np.float32)
    xh = xsc.astype(np.float16)
    xl = (xsc - xh.astype(np.float32)).astype(np.float16)
    # [T, N_CORES, P, 2, FDT]
    xh = xh.reshape(T, N_CORES, P, 1, FDT)
    xl = xl.reshape(T, N_CORES, P, 1, FDT)
    xpair = np.concatenate([xh, xl], axis=3)

    pi = np.arange(P)
    w_host = np.zeros((P, T, P), dtype=np.float32)
    diag_vals = (-(2.0 ** np.arange(T, dtype=np.float64)) * B0P).astype(np.float32)
    w_host[pi[:, None], np.arange(T)[None, :], pi[:, None]] = diag_vals[None, :]
    w_host = w_host.astype(ml_dtypes.bfloat16)
    w0_host = (-B0_EXACT * np.eye(P)).astype(np.float16)

    nc = _get_nc()
    in_maps = [
        {"xs": np.ascontiguousarray(xpair[:, c]).reshape(T, P, 2 * FDT),
         "wts": w_host, "w0": w0_host}
        for c in range(N_CORES)
    ]
    LAST_RESULTS = run_bass_kernel_spmd(nc, in_maps, list(range(N_CORES)))

    full = np.empty((T, N_CORES, P, FDT), dtype=np.float32)
    for c in range(N_CORES):
        res = LAST_RESULTS.results[c]
        y = np.asarray(res["out"])  # fp16 [T, P, FDT]
        full[:, c] = (y != 0).astype(np.float32)
    return full.reshape(T, B, D)


# revision 22
# speedup vs baseline: 1.1845x; 1.1845x over previous
"""CLIF spiking-neuron recurrence kernel for 8 Trainium2 NeuronCores.

Reference semantics (per element, T=64 sequential steps, gamma=0.5):
    u     = 0.5*u + x_t
    spike = (u >= 1.0)
    m     = s_prev * sigmoid(0.5*u) + spike
    s     = sigmoid(m)                       # carried (in-place sigmoid_)
    u     = u - spike*(1.0 + s)
Output: spikes [T, B, D] float32.

Design (vs the 3-group concat baseline at ~162us; this runs ~144us):
- 2 groups of 256 cols; per core [128 x 512] elements. V_g = 2^t*u in
  PSUM (half a bank per group); m in SBUF - no matmul ever touches m, so
  the M2 DVE write never orders against the PE accumulation group (the
  tile tracker is whole-tile for PSUM acc groups; learned the hard way).
- Critical cycle per group: sg-ACT (sigmoid of V) -> Y3 (DVE) -> reset
  matmul -> next sg-ACT. The m/s path (M2 -> s-ACT) runs off-cycle with
  a full step of slack.
- DVE: CLIF_Y3 (y = spike*quad(q), q = s_prev*sg, exact spike via the
  monotone sigmoid LUT compare sg >= c) and CLIF_M2 (m = q + spike).
- Both matmul kinds of step t share ONE stationary W_t = -B0P*2^t*I
  (bf16, exact): the reset (moving y fp16) and the x-add for t+1 (moving
  fp16 hi/lo pair of -2x/B0P, exact to ~2^-21) - weights stay loaded.
- Output is y (fp16, 128KB/step instead of 256KB): host computes
  spike = (y != 0), exact since y = spike*quad and quad >= 1.
"""

import sys
import types

import numpy as np
import ml_dtypes

# If BASS_TRACE is set but the image's antenv lacks axon_hooks,
# run_bass_kernel_spmd would crash importing it; install a null-hook
# module so tracing degrades gracefully instead.
try:
    import antenv.axon_hooks  # noqa: F401
except Exception:
    try:
        import antenv
        _hooks = types.ModuleType("antenv.axon_hooks")
        _hook_cell = [None]
        _hooks.set_axon_ntff_profile_hook = (
            lambda h: _hook_cell.__setitem__(0, h))
        _hooks.get_axon_ntff_profile_hook = lambda: _hook_cell[0]
        sys.modules["antenv.axon_hooks"] = _hooks
        antenv.axon_hooks = _hooks
    except Exception:
        pass

import concourse.bass as bass
import concourse.bacc as bacc
import concourse.mybir as mybir
import concourse.tile as tile
import concourse.dve_ops as dve_ops
from concourse.dve_spec import Spec, Src0, Src1, C0, C1, C2, One, lower, _has_src1
from concourse.dve_uop import DveOpSpec
from concourse.bass_utils import run_bass_kernel_spmd

F32 = mybir.dt.float32
BF16 = mybir.dt.bfloat16
FP16 = mybir.dt.float16
AF = mybir.ActivationFunctionType

T = 64
B = 128
D = 4096
N_CORES = 8
P = 128
NPC = B * D // N_CORES          # 65536 elements per core
FDT = NPC // P                  # 512 free columns per core
GW = 256                        # group width (2 groups)

# Constrained LS fit of f(q) = 1 + sigmoid(1 + q) on q in [0.30, 0.93]
# with constant term forced to B0P (for t>=1 spiking elements always have
# q >= 0.311 since s_prev >= 0.5 and sg >= c; t=0 uses a zero-s path with
# the exact-B0 stationary).
B0P = 1.734375                          # bf16-exact
B1C = 0.18530899
B2C = -0.03826911
YC1 = float(np.float32(B1C / B0P))
YC2 = float(np.float32(B2C / B0P))
B0_EXACT = float(np.float32(1.0 + 1.0 / (1.0 + np.exp(-1.0))))

_NC_CACHE = None
LAST_RESULTS = None


def _register_dve_op(name, spec):
    for op in dve_ops.OPS:
        if op.name == name:
            return op
    shas = {}
    for ver in ("v3", "v4"):
        u = lower(spec, ver=ver)
        shas[ver] = DveOpSpec(name=name, opcode=1, uops=u,
                              rd1_en=_has_src1(spec)).sha(ver)
    op = dve_ops.DveOp(name, spec, subdim=False, uops_sha=shas)
    dve_ops.OPS.append(op)
    dve_ops._SUB_OPCODE_FOR_NAME[name] = (
        dve_ops._CUSTOM_DVE_ROW_BASE + len(dve_ops.OPS) - 1)
    dve_ops.CUSTOM_DVE_SPECS[name] = spec
    return op


# y = (sg >= c) * ((C2*q + C1)*q + 1),  q = s_prev*sg: reset magnitude
# (1+sigmoid(1+q))/B0P as a quadratic of q. in0=s_prev, in1=sg, s0=c,
# s1=YC1, imm2=YC2. (Same op as the proven baseline CLIF_Y3.)
_q = Src0 * Src1
CLIF_Y3 = _register_dve_op("CLIF_Y3_ANT", Spec(
    body=(Src1 >= C0) * ((C2 * _q + C1) * _q + One),
    reference=lambda in0, in1, s0, s1, imm2:
        (in1 >= s0).astype(np.float32)
        * ((imm2 * (in0 * in1) + s1) * (in0 * in1) + 1.0),
))
# m = (s_prev*sg + (sg >= c)) * C1   (C1 = 1; fp32 to SBUF; the s-ACT
# reads it with scale 1 to produce s_t = sigmoid(m))
CLIF_M2 = _register_dve_op("CLIF_M2_ANT", Spec(
    body=(Src0 * Src1 + (Src1 >= C0)) * C1,
    reference=lambda in0, in1, s0, s1, imm2:
        (in0 * in1 + (in1 >= s0).astype(np.float32)) * s1,
))


def _build():
    nc = bacc.Bacc(None, target_bir_lowering=False, debug=False,
                   num_devices=N_CORES)

    # xs[t] = fp16 [hi(512) | lo(512)] pair of the scaled input (see host)
    xs = nc.declare_dram_parameter("xs", [T, P, 2 * FDT], FP16, isOutput=False)
    # wts[:, t, :] = -B0P * 2^t * I  (bf16, exact) for t >= 1
    wts = nc.declare_dram_parameter("wts", [P, T, P], BF16, isOutput=False)
    # w0 = -B0_EXACT * I (fp16)
    w0 = nc.declare_dram_parameter("w0", [P, P], FP16, isOutput=False)
    out = nc.declare_dram_parameter("out", [T, P, FDT], FP16, isOutput=True)

    with tile.TileContext(nc) as tc:
        with (
            tc.tile_pool(name="wpool", bufs=1) as wpool,
            tc.tile_pool(name="cpool", bufs=1) as cpool,
            tc.tile_pool(name="xpool", bufs=8) as xpool,
            tc.tile_pool(name="sgpool", bufs=8) as sgpool,
            tc.tile_pool(name="spool", bufs=6) as spool,
            tc.tile_pool(name="mppool", bufs=6) as mppool,
            tc.tile_pool(name="ypool", bufs=6) as ypool,
            tc.tile_pool(name="vpool", bufs=1, space="PSUM") as vpool,
        ):
            # --- one-time setup -------------------------------------------
            w0t = wpool.tile([P, P], FP16, tag="w0")
            nc.scalar.dma_start(w0t[:], w0[:])
            # Stream the 2MB of (mostly-zero diagonal) weights in 8-step
            # chunks: chunk 0 up front, the rest issued inside the loop a
            # full 8 steps ahead - keeps the prologue off the DMA critical
            # path (this was ~8us of startup stall when loaded up front).
            wtile = wpool.tile([P, T, P], BF16, tag="wts")
            nc.scalar.dma_start(wtile[:, 0:8, :], wts[:, 0:8, :])

            halft = cpool.tile([P, 1], F32, tag="half")
            nc.gpsimd.memset(halft[:], 0.5)
            ct = cpool.tile([P, 1], F32, tag="c")
            # c = sigmoid_LUT(0.5), same LUT as the per-step sigmoids
            nc.scalar.activation(ct[:], halft[:], AF.Sigmoid, bias=0.0, scale=1.0)
            c_ap = ct[:, 0:1]

            zs = cpool.tile([P, FDT], F32, tag="zs")   # zero s_prev for t=0
            nc.gpsimd.memset(zs[:], 0.0)

            # Per-group V state, half a PSUM bank each. m lives in SBUF
            # (mppool) - no matmul ever touches it, so the M2 DVE write
            # never orders against the PE accumulation group.
            V = [vpool.tile([P, GW], F32, name=f"V{g}", tag=f"V{g}")
                 for g in range(2)]

            # PE warm-up: dummy matmuls fill the otherwise-idle prologue
            # window so the HAM clock gate reaches 2.4 GHz before the first
            # real matmul
            junk = vpool.tile([P, 128], F32, tag="junk")
            for _ in range(10):
                nc.tensor.matmul(junk[:], w0t[:], w0t[:], start=True, stop=True,
                                 skip_group_check=True)

            # V_0 = W_0 @ x~_0  (x~_0 = -x_0/B0_EXACT)
            x0 = xpool.tile([P, 2, FDT], FP16, tag="x")
            nc.sync.dma_start(x0[:], xs[0])
            for g in range(2):
                o = g * GW
                nc.tensor.matmul(V[g][:], w0t[:], x0[:, 0, o:o + GW],
                                 start=True, stop=False, skip_group_check=True)
                nc.tensor.matmul(V[g][:], w0t[:], x0[:, 1, o:o + GW],
                                 start=False, stop=False, skip_group_check=True)

            s_prev = zs   # [P, FDT]: s_{t-1} for both groups (zero at t=0)

            # --- the recurrence -------------------------------------------
            for t in range(T):
                sc = float(2.0 ** (-t - 1))
                wsrc = w0t[:] if t == 0 else wtile[:, t, :]
                last = t == T - 1

                # input prefetch for the NEXT step (one wide DMA)
                if t < T - 1:
                    xnext = xpool.tile([P, 2, FDT], FP16, tag="x")
                    nc.sync.dma_start(xnext[:], xs[t + 1])
                # weight chunk for steps [8k, 8k+8), issued 8 steps ahead
                if t % 8 == 0 and t + 8 < T:
                    k = t + 8
                    nc.gpsimd.dma_start(wtile[:, k:k + 8, :], wts[:, k:k + 8, :])

                # sg = sigmoid(2^-(t+1) V) per group (critical-path ACT)
                sgw = []
                for g in range(2):
                    sg_g = sgpool.tile([P, GW], F32, name=f"sg{g}", tag=f"sg{g}")
                    nc.scalar.activation(sg_g[:], V[g][:],
                                         AF.Sigmoid, bias=0.0, scale=sc)
                    sgw.append(sg_g)

                yt = ypool.tile([P, FDT], FP16, tag="y")
                mp = mppool.tile([P, FDT], F32, tag="mp")
                st = spool.tile([P, FDT], F32, tag="st")

                # x-add for t+1 (early: fills PE, keeps the clock ramped).
                # One matmul per group: moving [2, GW] (hi|lo pair), output
                # visits V_g twice via a stride-0 broadcast AP - the PSUM
                # accumulate adds both passes.
                if not last:
                    for g in range(2):
                        o = g * GW
                        vb = V[g][:].unsqueeze(1).broadcast_to([P, 2, GW])
                        nc.tensor.matmul(vb, wsrc, xnext[:, :, o:o + GW],
                                         start=False, stop=False,
                                         skip_group_check=True)

                # DVE order: Y3_g0, M2_g0, Y3_g1, M2_g1 (keeps group 0's
                # whole chain ahead of group 1's)
                for g in range(2):
                    o = g * GW
                    nc.vector._custom_dve(CLIF_Y3, out=yt[:, o:o + GW],
                                          in0=s_prev[:, o:o + GW],
                                          in1=sgw[g][:],
                                          s0=c_ap, s1=YC1, imm2=YC2)
                    if not last:
                        # reset: V_g += W_t @ y_g
                        nc.tensor.matmul(V[g][:], wsrc, yt[:, o:o + GW],
                                         start=False, stop=(t == T - 2),
                                         skip_group_check=True)
                        # m = q + spike (fp32, SBUF)
                        nc.vector._custom_dve(CLIF_M2, out=mp[:, o:o + GW],
                                              in0=s_prev[:, o:o + GW],
                                              in1=sgw[g][:],
                                              s0=c_ap, s1=1.0)
                        # s_t = sigmoid(m) for the next step (off-cycle ACT)
                        nc.scalar.activation(st[:, o:o + GW], mp[:, o:o + GW],
                                             AF.Sigmoid, bias=0.0, scale=1.0)

                # output y_t (fp16); host: spike = (y != 0)
                nc.gpsimd.dma_start(out[t], yt[:])
                if not last:
                    s_prev = st

    nc.compile()
    return nc


def _get_nc():
    global _NC_CACHE
    if _NC_CACHE is None:
        _NC_CACHE = _build()
    return _NC_CACHE


def kernel(x_seq: np.ndarray) -> np.ndarray:
    global LAST_RESULTS
    x = np.ascontiguousarray(x_seq, dtype=np.float32)
    assert x.shape == (T, B, D), x.shape

    # Host prep: x~_t scaled so that the matmul under the stationary W
    # current at issue time yields exactly the 2^t * x_t that V needs
    # (V_t = V_{t-1} + 2^t * x_t):
    #   t=0:  V_0 = W_0 @ x~_0,      W_0 = -B0_EXACT*I -> x~_0 = -x/B0_EXACT
    #   t=1:  issued under W_0                         -> x~_1 = -2x/B0_EXACT
    #   t>=2: issued under W_{t-1} = -B0P*2^{t-1}*I    -> x~_t = -2x/B0P
    xf = x.reshape(T, -1).astype(np.float64)
    scale = np.empty((T, 1), dtype=np.float64)
    scale[0, 0] = -1.0 / B0_EXACT
    scale[1, 0] = -2.0 / B0_EXACT
    scale[2:, 0] = -2.0 / B0P
    xsc = (xf * scale).astype(np.float32)
    xh = xsc.astype(np.float16)
    xl = (xsc - xh.astype(np.float32)).astype(np.float16)
    # [T, N_CORES, P, 2, FDT]
    xh = xh.reshape(T, N_CORES, P, 1, FDT)
    xl = xl.reshape(T, N_CORES, P, 1, FDT)
    xpair = np.concatenate([xh, xl], axis=3)

    pi = np.arange(P)
    w_host = np.zeros((P, T, P), dtype=np.float32)
    diag_vals = (-(2.0 ** np.arange(T, dtype=np.float64)) * B0P).astype(np.float32)
    w_host[pi[:, None], np.arange(T)[None, :], pi[:, None]] = diag_vals[None, :]
    w_host = w_host.astype(ml_dtypes.bfloat16)
    w0_host = (-B0_EXACT * np.eye(P)).astype(np.float16)

    nc = _get_nc()
    in_maps = [
        {"xs": np.ascontiguousarray(xpair[:, c]).reshape(T, P, 2 * FDT),
         "wts": w_host, "w0": w0_host}
        for c in range(N_CORES)
    ]
    LAST_RESULTS = run_bass_kernel_spmd(nc, in_maps, list(range(N_CORES)))

    full = np.empty((T, N_CORES, P, FDT), dtype=np.float32)
    for c in range(N_CORES):
        res = LAST_RESULTS.results[c]
        y = np.asarray(res["out"])  # fp16 [T, P, FDT]
        full[:, c] = (y != 0).astype(np.float32)
    return full.reshape(T, B, D)


# revision 23
# speedup vs baseline: 1.1892x; 1.0040x over previous
"""CLIF spiking-neuron recurrence kernel for 8 Trainium2 NeuronCores.

Reference semantics (per element, T=64 sequential steps, gamma=0.5):
    u     = 0.5*u + x_t
    spike = (u >= 1.0)
    m     = s_prev * sigmoid(0.5*u) + spike
    s     = sigmoid(m)                       # carried (in-place sigmoid_)
    u     = u - spike*(1.0 + s)
Output: spikes [T, B, D] float32.

Design (vs the 3-group concat baseline at ~162us; this runs ~144us):
- 2 groups of 256 cols; per core [128 x 512] elements. V_g = 2^t*u in
  PSUM (half a bank per group); m in SBUF - no matmul ever touches m, so
  the M2 DVE write never orders against the PE accumulation group (the
  tile tracker is whole-tile for PSUM acc groups; learned the hard way).
- Critical cycle per group: sg-ACT (sigmoid of V) -> Y3 (DVE) -> reset
  matmul -> next sg-ACT. The m/s path (M2 -> s-ACT) runs off-cycle with
  a full step of slack.
- DVE: CLIF_Y3 (y = spike*quad(q), q = s_prev*sg, exact spike via the
  monotone sigmoid LUT compare sg >= c) and CLIF_M2 (m = q + spike).
- Both matmul kinds of step t share ONE stationary W_t = -B0P*2^t*I
  (bf16, exact): the reset (moving y fp16) and the x-add for t+1 (moving
  fp16 hi/lo pair of -2x/B0P, exact to ~2^-21) - weights stay loaded.
- Output is y (fp16, 128KB/step instead of 256KB): host computes
  spike = (y != 0), exact since y = spike*quad and quad >= 1.
"""

import sys
import types

import numpy as np
import ml_dtypes

# If BASS_TRACE is set but the image's antenv lacks axon_hooks,
# run_bass_kernel_spmd would crash importing it; install a null-hook
# module so tracing degrades gracefully instead.
try:
    import antenv.axon_hooks  # noqa: F401
except Exception:
    try:
        import antenv
        _hooks = types.ModuleType("antenv.axon_hooks")
        _hook_cell = [None]
        _hooks.set_axon_ntff_profile_hook = (
            lambda h: _hook_cell.__setitem__(0, h))
        _hooks.get_axon_ntff_profile_hook = lambda: _hook_cell[0]
        sys.modules["antenv.axon_hooks"] = _hooks
        antenv.axon_hooks = _hooks
    except Exception:
        pass

import concourse.bass as bass
import concourse.bacc as bacc
import concourse.mybir as mybir
import concourse.tile as tile
import concourse.dve_ops as dve_ops
from concourse.dve_spec import Spec, Src0, Src1, C0, C1, C2, One, lower, _has_src1
from concourse.dve_uop import DveOpSpec
from concourse.bass_utils import run_bass_kernel_spmd

F32 = mybir.dt.float32
BF16 = mybir.dt.bfloat16
FP16 = mybir.dt.float16
AF = mybir.ActivationFunctionType

T = 64
B = 128
D = 4096
N_CORES = 8
P = 128
NPC = B * D // N_CORES          # 65536 elements per core
FDT = NPC // P                  # 512 free columns per core
GW = 256                        # group width (2 groups)

# Constrained LS fit of f(q) = 1 + sigmoid(1 + q) on q in [0.30, 0.93]
# with constant term forced to B0P (for t>=1 spiking elements always have
# q >= 0.311 since s_prev >= 0.5 and sg >= c; t=0 uses a zero-s path with
# the exact-B0 stationary).
B0P = 1.734375                          # bf16-exact
B1C = 0.18530899
B2C = -0.03826911
YC1 = float(np.float32(B1C / B0P))
YC2 = float(np.float32(B2C / B0P))
B0_EXACT = float(np.float32(1.0 + 1.0 / (1.0 + np.exp(-1.0))))

_NC_CACHE = None
LAST_RESULTS = None


def _register_dve_op(name, spec):
    for op in dve_ops.OPS:
        if op.name == name:
            return op
    shas = {}
    for ver in ("v3", "v4"):
        u = lower(spec, ver=ver)
        shas[ver] = DveOpSpec(name=name, opcode=1, uops=u,
                              rd1_en=_has_src1(spec)).sha(ver)
    op = dve_ops.DveOp(name, spec, subdim=False, uops_sha=shas)
    dve_ops.OPS.append(op)
    dve_ops._SUB_OPCODE_FOR_NAME[name] = (
        dve_ops._CUSTOM_DVE_ROW_BASE + len(dve_ops.OPS) - 1)
    dve_ops.CUSTOM_DVE_SPECS[name] = spec
    return op


# y = (sg >= c) * ((C2*q + C1)*q + 1),  q = s_prev*sg: reset magnitude
# (1+sigmoid(1+q))/B0P as a quadratic of q. in0=s_prev, in1=sg, s0=c,
# s1=YC1, imm2=YC2. (Same op as the proven baseline CLIF_Y3.)
_q = Src0 * Src1
CLIF_Y3 = _register_dve_op("CLIF_Y3_ANT", Spec(
    body=(Src1 >= C0) * ((C2 * _q + C1) * _q + One),
    reference=lambda in0, in1, s0, s1, imm2:
        (in1 >= s0).astype(np.float32)
        * ((imm2 * (in0 * in1) + s1) * (in0 * in1) + 1.0),
))
# m = (s_prev*sg + (sg >= c)) * C1   (C1 = 1; fp32 to SBUF; the s-ACT
# reads it with scale 1 to produce s_t = sigmoid(m))
CLIF_M2 = _register_dve_op("CLIF_M2_ANT", Spec(
    body=(Src0 * Src1 + (Src1 >= C0)) * C1,
    reference=lambda in0, in1, s0, s1, imm2:
        (in0 * in1 + (in1 >= s0).astype(np.float32)) * s1,
))


def _build():
    nc = bacc.Bacc(None, target_bir_lowering=False, debug=False,
                   num_devices=N_CORES)

    # xs[t] = fp16 [hi(512) | lo(512)] pair of the scaled input (see host)
    xs = nc.declare_dram_parameter("xs", [T, P, 2 * FDT], FP16, isOutput=False)
    # wts[:, t, :] = -B0P * 2^t * I  (bf16, exact) for t >= 1
    wts = nc.declare_dram_parameter("wts", [P, T, P], BF16, isOutput=False)
    # w0 = -B0_EXACT * I (fp16)
    w0 = nc.declare_dram_parameter("w0", [P, P], FP16, isOutput=False)
    out = nc.declare_dram_parameter("out", [T, P, FDT], FP16, isOutput=True)

    with tile.TileContext(nc) as tc:
        with (
            tc.tile_pool(name="wpool", bufs=1) as wpool,
            tc.tile_pool(name="cpool", bufs=1) as cpool,
            tc.tile_pool(name="xpool", bufs=8) as xpool,
            tc.tile_pool(name="sgpool", bufs=8) as sgpool,
            tc.tile_pool(name="spool", bufs=6) as spool,
            tc.tile_pool(name="mppool", bufs=6) as mppool,
            tc.tile_pool(name="ypool", bufs=6) as ypool,
            tc.tile_pool(name="vpool", bufs=1, space="PSUM") as vpool,
        ):
            # --- one-time setup -------------------------------------------
            # x0 first: it gates V_0 and hence the whole pipeline ramp
            x0 = xpool.tile([P, 2, FDT], FP16, tag="x")
            nc.sync.dma_start(x0[:], xs[0])
            w0t = wpool.tile([P, P], FP16, tag="w0")
            nc.scalar.dma_start(w0t[:], w0[:])
            # Stream the 2MB of (mostly-zero diagonal) weights in 8-step
            # chunks: chunk 0 up front, the rest issued inside the loop a
            # full 8 steps ahead - keeps the prologue off the DMA critical
            # path (this was ~8us of startup stall when loaded up front).
            wtile = wpool.tile([P, T, P], BF16, tag="wts")
            nc.scalar.dma_start(wtile[:, 0:8, :], wts[:, 0:8, :])

            halft = cpool.tile([P, 1], F32, tag="half")
            nc.gpsimd.memset(halft[:], 0.5)
            ct = cpool.tile([P, 1], F32, tag="c")
            # c = sigmoid_LUT(0.5), same LUT as the per-step sigmoids
            nc.scalar.activation(ct[:], halft[:], AF.Sigmoid, bias=0.0, scale=1.0)
            c_ap = ct[:, 0:1]

            zs = cpool.tile([P, FDT], F32, tag="zs")   # zero s_prev for t=0
            nc.gpsimd.memset(zs[:], 0.0)

            # Per-group V state, half a PSUM bank each. m lives in SBUF
            # (mppool) - no matmul ever touches it, so the M2 DVE write
            # never orders against the PE accumulation group.
            V = [vpool.tile([P, GW], F32, name=f"V{g}", tag=f"V{g}")
                 for g in range(2)]

            # PE warm-up: dummy matmuls fill the otherwise-idle prologue
            # window so the HAM clock gate reaches 2.4 GHz before the first
            # real matmul
            junk = vpool.tile([P, 128], F32, tag="junk")
            for _ in range(6):
                nc.tensor.matmul(junk[:], w0t[:], w0t[:], start=True, stop=True,
                                 skip_group_check=True)

            # V_0 = W_0 @ x~_0  (x~_0 = -x_0/B0_EXACT)
            for g in range(2):
                o = g * GW
                nc.tensor.matmul(V[g][:], w0t[:], x0[:, 0, o:o + GW],
                                 start=True, stop=False, skip_group_check=True)
                nc.tensor.matmul(V[g][:], w0t[:], x0[:, 1, o:o + GW],
                                 start=False, stop=False, skip_group_check=True)

            s_prev = zs   # [P, FDT]: s_{t-1} for both groups (zero at t=0)

            # --- the recurrence -------------------------------------------
            for t in range(T):
                sc = float(2.0 ** (-t - 1))
                wsrc = w0t[:] if t == 0 else wtile[:, t, :]
                last = t == T - 1

                # input prefetch for the NEXT step (one wide DMA)
                if t < T - 1:
                    xnext = xpool.tile([P, 2, FDT], FP16, tag="x")
                    nc.sync.dma_start(xnext[:], xs[t + 1])
                # weight chunk for steps [8k, 8k+8), issued 8 steps ahead
                if t % 8 == 0 and t + 8 < T:
                    k = t + 8
                    nc.gpsimd.dma_start(wtile[:, k:k + 8, :], wts[:, k:k + 8, :])

                # sg = sigmoid(2^-(t+1) V) per group (critical-path ACT)
                sgw = []
                for g in range(2):
                    sg_g = sgpool.tile([P, GW], F32, name=f"sg{g}", tag=f"sg{g}")
                    nc.scalar.activation(sg_g[:], V[g][:],
                                         AF.Sigmoid, bias=0.0, scale=sc)
                    sgw.append(sg_g)

                yt = ypool.tile([P, FDT], FP16, tag="y")
                mp = mppool.tile([P, FDT], F32, tag="mp")
                st = spool.tile([P, FDT], F32, tag="st")

                # x-add for t+1 (early: fills PE, keeps the clock ramped).
                # One matmul per group: moving [2, GW] (hi|lo pair), output
                # visits V_g twice via a stride-0 broadcast AP - the PSUM
                # accumulate adds both passes.
                if not last:
                    for g in range(2):
                        o = g * GW
                        vb = V[g][:].unsqueeze(1).broadcast_to([P, 2, GW])
                        nc.tensor.matmul(vb, wsrc, xnext[:, :, o:o + GW],
                                         start=False, stop=False,
                                         skip_group_check=True)

                # DVE order: Y3_g0, M2_g0, Y3_g1, M2_g1 (keeps group 0's
                # whole chain ahead of group 1's)
                for g in range(2):
                    o = g * GW
                    nc.vector._custom_dve(CLIF_Y3, out=yt[:, o:o + GW],
                                          in0=s_prev[:, o:o + GW],
                                          in1=sgw[g][:],
                                          s0=c_ap, s1=YC1, imm2=YC2)
                    if not last:
                        # reset: V_g += W_t @ y_g
                        nc.tensor.matmul(V[g][:], wsrc, yt[:, o:o + GW],
                                         start=False, stop=(t == T - 2),
                                         skip_group_check=True)
                        # m = q + spike (fp32, SBUF)
                        nc.vector._custom_dve(CLIF_M2, out=mp[:, o:o + GW],
                                              in0=s_prev[:, o:o + GW],
                                              in1=sgw[g][:],
                                              s0=c_ap, s1=1.0)
                        # s_t = sigmoid(m) for the next step (off-cycle ACT)
                        nc.scalar.activation(st[:, o:o + GW], mp[:, o:o + GW],
                                             AF.Sigmoid, bias=0.0, scale=1.0)

                # output y_t (fp16); host: spike = (y != 0)
                nc.gpsimd.dma_start(out[t], yt[:])
                if not last:
                    s_prev = st

    nc.compile()
    return nc


def _get_nc():
    global _NC_CACHE
    if _NC_CACHE is None:
        _NC_CACHE = _build()
    return _NC_CACHE


def kernel(x_seq: np.ndarray) -> np.ndarray:
    global LAST_RESULTS
    x = np.ascontiguousarray(x_seq, dtype=np.float32)
    assert x.shape == (T, B, D), x.shape

    # Host prep: x~_t scaled so that the matmul under the stationary W
    # current at issue time yields exactly the 2^t * x_t that V needs
    # (V_t = V_{t-1} + 2^t * x_t):
    #   t=0:  V_0 = W_0 @ x~_0,      W_0 = -B0_EXACT*I -> x~_0 = -x/B0_EXACT
    #   t=1:  issued under W_0                         -> x~_1 = -2x/B0_EXACT
    #   t>=2: issued under W_{t-1} = -B0P*2^{t-1}*I    -> x~_t = -2x/B0P
    xf = x.reshape(T, -1).astype(np.float64)
    scale = np.empty((T, 1), dtype=np.float64)
    scale[0, 0] = -1.0 / B0_EXACT
    scale[1, 0] = -2.0 / B0_EXACT
    scale[2:, 0] = -2.0 / B0P
    xsc = (xf * scale).astype(np.float32)
    xh = xsc.astype(np.float16)
    xl = (xsc - xh.astype(np.float32)).astype(np.float16)
    # [T, N_CORES, P, 2, FDT]
    xh = xh.reshape(T, N_CORES, P, 1, FDT)
    xl = xl.reshape(T, N_CORES, P, 1, FDT)
    xpair = np.concatenate([xh, xl], axis=3)

    pi = np.arange(P)
    w_host = np.zeros((P, T, P), dtype=np.float32)
    diag_vals = (-(2.0 ** np.arange(T, dtype=np.float64)) * B0P).astype(np.float32)
    w_host[pi[:, None], np.arange(T)[None, :], pi[:, None]] = diag_vals[None, :]
    w_host = w_host.astype(ml_dtypes.bfloat16)
    w0_host = (-B0_EXACT * np.eye(P)).astype(np.float16)

    nc = _get_nc()
    in_maps = [
        {"xs": np.ascontiguousarray(xpair[:, c]).reshape(T, P, 2 * FDT),
         "wts": w_host, "w0": w0_host}
        for c in range(N_CORES)
    ]
    LAST_RESULTS = run_bass_kernel_spmd(nc, in_maps, list(range(N_CORES)))

    full = np.empty((T, N_CORES, P, FDT), dtype=np.float32)
    for c in range(N_CORES):
        res = LAST_RESULTS.results[c]
        y = np.asarray(res["out"])  # fp16 [T, P, FDT]
        full[:, c] = (y != 0).astype(np.float32)
    return full.reshape(T, B, D)
